# revision 81
# baseline (speedup 1.0000x reference)
"""Trainium2 Bass kernel: 8-head attention block (BN-folded projections,
relative-position bias, softmax, GELU + output projection).

Sharding: data-parallel over batch across 8 NeuronCores (2 batch elems/core).
All weights / bias tables replicated; no collectives.

Engine-balanced design for the TimelineSim cost model (matmuls charge
output free-size on PE.ENGINE; ACT charges free-size + ~143ns/instr;
engines execute their instruction streams in order):

  ACT is the pacing engine: exp of the 2*8*1024*1024 attention logits.
  To cut per-instruction overhead, dots psums are written as a global
  CHUNK STREAM: [128,512] chunks packed 3-per-psum-tile ([128,3,512],
  double-buffered = 6 of 8 PSUM banks), so each exp instruction covers
  1536 elements (86 exps instead of 128). E-mults still run per
  (jt, b) slot on DVE (bf16 2x), slicing across exp tiles when a slot
  straddles a tile boundary.
  - relative-position bias applied as a POST-EXP multiply by the
    precomputed table E = exp(bias/scale) on DVE.
  - softmax denominators come free from AV: stationary is V augmented
    with 64 ones columns, so the AV matmul stream yields AV on
    partitions 0:64 and the softmax sums on 64:128.
  - AV runs GROUP-SERIAL ((b,isl) groups of 8 accumulation matmuls on
    2 rotating PSUM banks), each group normalized (DVE recip + mult,
    partition-shifted) right at its stop matmul.
  - software pipelining: head h's AV groups are emitted inside head
    h+1's dots/exp loop; V projection and dt=1 Q/K projections ride
    the h0 loop as extra units; half the V psum->sbuf copies go
    through ACT (Copy shares the Exp table set) to fill PE-paced gaps.
  - out-projection runs in it-PAIRS: two accumulation groups write
    disjoint column halves of one PSUM bank (start=True only touches
    the addresses each group writes), so one bias-TT and one DMA
    serve two units -- halving the DVE TT serialization that paces
    the tail. Psums alternate between the avp pool and the dpsum
    pool (free after the last exp) so 4 pairs are in flight.
  - wqk[0] rides the ACT DMA queue so the first projection's inputs
    land a queue-slot earlier; everything else streams on SP.

  HW workaround: tile_position (96,0) is fatal (quadrant-3 bug), so
  heads 3/7's dots run as K=64 matmuls at (64,0) against a KT copy
  with the neighbouring head's rows zeroed.

  HW legality notes (the BIR verifier rejects, cost model accepts):
  AluOpType.divide on DVE TensorTensor; keep reciprocal+mult.
"""

import os
import numpy as np
import ml_dtypes

import concourse.bass as bass
import concourse.tile as tile
from concourse import bacc, mybir
from concourse.bass_utils import run_bass_kernel_spmd
from concourse.tile import add_dep_helper

NPBF16 = ml_dtypes.bfloat16
BF16 = mybir.dt.bfloat16
F32 = mybir.dt.float32

HEADS, DK, DV = 8, 32, 64
N = 1024          # positions = 32*32
C = 256           # channels
IDV = HEADS * DV  # 512
NCORES = 8
BLOC = 2          # batch elems per core
SCALE = float(DK) ** -0.5
EPS = 1e-5

_CACHE = {}


def _build_nc():
    nc = bacc.Bacc("TRN2", target_bir_lowering=False, debug=False)

    xt_d = nc.declare_dram_parameter("xt", [BLOC, 2, 128, N], BF16, isOutput=False)
    wqk_d = nc.declare_dram_parameter("wqk", [2, 128, 2, 2, 128], BF16,
                                      isOutput=False)
    wv_d = nc.declare_dram_parameter("wv", [128, 2, IDV], BF16, isOutput=False)
    wo_d = nc.declare_dram_parameter("wo", [128, 4, C], BF16, isOutput=False)
    oqk_d = nc.declare_dram_parameter("oqk", [128, 2, 2], F32, isOutput=False)
    ovg_d = nc.declare_dram_parameter("ovg", [128, 4], F32, isOutput=False)
    bout_d = nc.declare_dram_parameter("bout", [128, 2 * C], F32,
                                       isOutput=False)
    # E[h, jt, j1, i] = exp(pos_bias[j, i, h] / SCALE)
    e_d = nc.declare_dram_parameter("etab", [HEADS, 8, 128, N], BF16,
                                    isOutput=False)
    out_d = nc.declare_dram_parameter("out", [BLOC, N, C], BF16, isOutput=True)

    Exp = mybir.ActivationFunctionType.Exp
    Gelu = mybir.ActivationFunctionType.Gelu

    with tile.TileContext(nc) as tc:
        with (
            tc.tile_pool(name="const", bufs=1) as const,
            tc.tile_pool(name="persist", bufs=1) as persist,
            tc.tile_pool(name="ep", bufs=4) as ep,        # E tiles in flight
            tc.tile_pool(name="etp", bufs=8) as etp,      # raw exp tiles
            tc.tile_pool(name="et2p", bufs=22) as et2p,   # attn-weight tiles
            tc.tile_pool(name="recp", bufs=3) as recp,
            tc.tile_pool(name="dpsum", bufs=2, space="PSUM") as dpsum,
            tc.tile_pool(name="avp", bufs=2, space="PSUM") as avp,
        ):
            dma = nc.sync

            # critical-path DMAs first: b0's x, then the Q/K weights the
            # first dots need, then everything else.
            xt = {}
            for b in range(BLOC):
                for ct in range(2):
                    xt[b, ct] = persist.tile([128, N], BF16, tag=f"xt{b}{ct}",
                                             name=f"xt{b}{ct}")
            wqk_s = const.tile([128, 2, 2, 2, 128], BF16, tag="wqk")
            nc.scalar.dma_start(wqk_s[:, 0], wqk_d[0])
            dma.dma_start(xt[0, 0][:], xt_d[0, 0])
            dma.dma_start(xt[0, 1][:], xt_d[0, 1])
            oqk_s = const.tile([128, 2, 2], F32, tag="oqk")
            dma.dma_start(oqk_s[:], oqk_d[:])
            dma.dma_start(xt[1, 0][:], xt_d[1, 0])
            dma.dma_start(xt[1, 1][:], xt_d[1, 1])
            dma.dma_start(wqk_s[:, 1], wqk_d[1])
            wv_s = const.tile([128, 2, IDV], BF16, tag="wv")
            dma.dma_start(wv_s[:], wv_d[:])
            wo_s = const.tile([128, 4, C], BF16, tag="wo")
            dma.dma_start(wo_s[:], wo_d[:])
            ovg_s = const.tile([128, 4], F32, tag="ovg")
            dma.dma_start(ovg_s[:], ovg_d[:])
            bout_s = const.tile([128, 2 * C], F32, tag="bout")
            dma.dma_start(bout_s[:], bout_d[:])


            qt, kt, kzt, vt = {}, {}, {}, {}
            for b in range(BLOC):
                for dt in range(2):
                    qt[b, dt] = persist.tile([128, N], BF16, tag=f"qt{b}{dt}",
                                             name=f"qt{b}{dt}")
                    kt[b, dt] = persist.tile([128, N], BF16, tag=f"kt{b}{dt}",
                                             name=f"kt{b}{dt}")
                    kzt[b, dt] = persist.tile([128, N], BF16, tag=f"kz{b}{dt}",
                                              name=f"kz{b}{dt}")

            def qk_unit(b, dt, which):
                """One Q-or-K projection tile: 4 matmuls + DVE assembly.

                Uses the avp psum pool (idle during startup/h0) so the
                dots double-buffer in dpsum is never blocked."""
                for ih in range(2):
                    qk_half(b, dt, which, ih)

            def qk_half(b, dt, which, ih):
                w = 0 if which == "q" else 1
                dst = qt[b, dt] if which == "q" else kt[b, dt]
                ps = avp.tile([128, 512], F32, tag="av",
                              name=f"pps{b}{dt}{which}{ih}")
                for ct in range(2):
                    nc.tensor.matmul(
                        ps[:],
                        wqk_s[:, dt, w, ct, :],
                        xt[b, ct][:, ih * 512:(ih + 1) * 512],
                        start=(ct == 0), stop=(ct == 1),
                    )
                nc.vector.tensor_scalar_add(
                    dst[:, ih * 512:(ih + 1) * 512], ps[:],
                    oqk_s[:, w, dt:dt + 1])

            def kz_unit(b, dt):
                """kz = KT with head-2 rows zeroed (cheap all-SBUF copy);
                only needed once heads 3 (dt0) / 7 (dt1) come up."""
                kz = kzt[b, dt]
                nc.gpsimd.memset(kz[64:96, :], 0.0)
                nc.gpsimd.tensor_copy(kz[96:128, :], kt[b, dt][96:128, :])

            def v_unit(b, jt, act_copy=False):
                """One Vaug tile: [j, (h, V|ones 128)]; cols 64:128 = 1.0.

                act_copy routes the psum->sbuf copy through the ACT engine
                (Copy shares the Exp table set, so no table loads); used
                for half the tiles to relieve DVE during startup."""
                v = persist.tile([128, HEADS, 128], BF16, tag=f"v{b}{jt}",
                                 name=f"v{b}{jt}")
                vt[b, jt] = v
                nc.gpsimd.memset(v[:, :, 64:128], 1.0)
                ps = avp.tile([128, 512], F32, tag="av", name=f"vps{b}{jt}")
                for ct in range(2):
                    nc.tensor.matmul(
                        ps[:],
                        xt[b, ct][:, jt * 128:(jt + 1) * 128],
                        wv_s[:, ct, :],
                        start=(ct == 0), stop=(ct == 1),
                    )
                if act_copy:
                    nc.scalar.activation(
                        v[:, :, 0:64],
                        ps[:].rearrange("p (h d) -> p h d", h=HEADS),
                        mybir.ActivationFunctionType.Copy)
                else:
                    nc.vector.tensor_copy(
                        v[:, :, 0:64],
                        ps[:].rearrange("p (h d) -> p h d", h=HEADS))

            # warm the PE p-state during the initial DMA wait: ~3us of
            # matmuls on a zeroed scratch (results never read).
            wscr = persist.tile([128, 512], BF16, tag="wscr", name="wscr")
            nc.gpsimd.memset(wscr[:], 0.0)
            for w in range(6):
                wps = avp.tile([128, 512], F32, tag="av", name=f"warm{w}")
                nc.tensor.matmul(wps[:], wscr[:, 0:128], wscr[:],
                                 start=True, stop=True)

            # E-table quad DMAs, kept 2 in flight ahead of consumption
            equads = [(h, q) for h in range(HEADS) for q in range(2)]
            etabs = {}
            eptr = [0]

            def issue_equad():
                if eptr[0] >= len(equads):
                    return
                h, q = equads[eptr[0]]
                eptr[0] += 1
                t = ep.tile([128, 4, N], BF16, tag="etab", name=f"e{h}{q}")
                etabs[h, q] = t
                dma.dma_start(
                    t[:], e_d[h, 4 * q:4 * q + 4].rearrange("t j i -> j t i"))

            issue_equad()
            issue_equad()

            # b0's dt=0 projections up front (head 0 starts on them); all
            # other projection work rides head 0's loop as extra units.
            qk_half(0, 0, "q", 0)
            qk_half(0, 0, "k", 0)
            qk_half(0, 0, "q", 1)
            qk_half(0, 0, "k", 1)
            extras = [lambda: qk_unit(1, 0, "q"), lambda: qk_unit(1, 0, "k")]
            for b in range(BLOC):
                for jt in range(8):
                    extras.append(lambda b=b, jt=jt: v_unit(b, jt, jt % 2 == 0))
            for b in range(BLOC):
                extras.append(lambda b=b: qk_unit(b, 1, "q"))
                extras.append(lambda b=b: qk_unit(b, 1, "k"))
            for b in range(BLOC):
                for dt in range(2):
                    extras.append(lambda b=b, dt=dt: kz_unit(b, dt))
            extras = extras[::-1]

            # gt[b, hp]: gelu-layout attention output [(2 heads x 64 d), i]
            gt = {}
            for b in range(BLOC):
                for hp in range(4):
                    gt[b, hp] = persist.tile([128, N], BF16, tag=f"g{b}{hp}",
                                             name=f"g{b}{hp}")

            last_exp = [None]
            avt = {}

            def emit_av_piece(h, b, isl, jt, et2_prev):
                """One AV+sums accumulation matmul; group = (h,b,isl) over
                8 jt chunks, normalized immediately at the stop matmul."""
                if (h, b, isl) not in avt:
                    avt[h, b, isl] = avp.tile([128, 512], F32, tag="av",
                                              name=f"av{h}{b}{isl}")
                nc.tensor.matmul(
                    avt[h, b, isl][:],
                    vt[b, jt][:, h, :],
                    et2_prev[jt, b][:, isl * 512:(isl + 1) * 512],
                    start=(jt == 0), stop=(jt == 7),
                )
                if jt == 7:
                    emit_norm_piece(h, b, isl)
                    del avt[h, b, isl]

            def emit_norm_piece(h, b, isl):
                """softmax divide: gt rows = av(0:64) * 1/sums(64:128)."""
                tp = avt[h, b, isl]
                rec = recp.tile([64, 512], F32, tag="rec",
                                name=f"rec{h}{b}{isl}")
                nc.vector.reciprocal(rec[:], tp[64:128, :])
                nc.vector.tensor_tensor(
                    gt[b, h // 2][64 * (h % 2):64 * (h % 2) + 64,
                                  isl * 512:(isl + 1) * 512],
                    tp[0:64, :], rec[:],
                    mybir.AluOpType.mult)

            # ---- chunk-stream dots/exp machinery: dots psums are written as
            # a global stream of [128,512] chunks packed 3-per-psum-tile, so
            # each exp instruction covers 1536 elems (fewer ACT fixed costs).
            # E-mults still run per (jt, b) slot, slicing across exp tiles.
            cstate = {
                "c": 0,          # next chunk index
                "tile": None,    # current dps tile
                "et": None,      # current et tile
                "base": 0,       # chunk index of current tile's pos 0
                "exp_c": 0,      # chunks covered by emitted exps
                "pend": [],      # pending E-mult slots
            }
            NCHUNK = 3

            def dots_chunk(h, jt, b, ih):
                """Write one [128,512] dots chunk into the stream."""
                dt, hq = h // 4, h % 4
                pos = cstate["c"] % NCHUNK
                if pos == 0:
                    cstate["tile"] = dpsum.tile([128, NCHUNK, 512], F32,
                                                tag="dps",
                                                name=f"dps{cstate['c']}")
                    cstate["et"] = etp.tile([128, NCHUNK, 512], BF16,
                                            tag="et", name=f"et{cstate['c']}")
                    cstate["base"] = cstate["c"]
                dps = cstate["tile"]
                if hq < 3:
                    nc.tensor.matmul(
                        dps[:, pos, :],
                        kt[b, dt][32 * hq:32 * hq + 32,
                                  jt * 128:(jt + 1) * 128],
                        qt[b, dt][32 * hq:32 * hq + 32,
                                  ih * 512:(ih + 1) * 512],
                        start=True, stop=True,
                        tile_position=(32 * hq, 0),
                    )
                else:
                    nc.tensor.matmul(
                        dps[:, pos, :],
                        kzt[b, dt][64:128, jt * 128:(jt + 1) * 128],
                        qt[b, dt][64:128, ih * 512:(ih + 1) * 512],
                        start=True, stop=True,
                        tile_position=(64, 0),
                    )
                ref = (cstate["et"], pos)
                cstate["c"] += 1
                if cstate["c"] - cstate["base"] == NCHUNK:
                    flush_exp()
                return ref

            def flush_exp():
                """Exp the current (possibly partial) dps tile."""
                ntile = cstate["c"] - cstate["base"]
                if cstate["tile"] is None or ntile == 0:
                    return
                nc.scalar.activation(cstate["et"][:, 0:ntile, :],
                                     cstate["tile"][:, 0:ntile, :],
                                     Exp, scale=SCALE)
                cstate["exp_c"] = cstate["c"]
                cstate["tile"] = None
                drain_mults()

            mult_done = set()

            def drain_mults():
                pend = cstate["pend"]
                while pend and pend[0][4][-1][2] < cstate["exp_c"]:
                    h, jt, b, t2, refs = pend.pop(0)
                    mult_done.add((h, jt, b))
                    qd = (h, jt // 4)
                    etq = etabs[qd]
                    if refs[0][0] is refs[1][0]:
                        et_t, p0 = refs[0][0], refs[0][1]
                        nc.vector.tensor_tensor(
                            t2[:].rearrange("p (a f) -> p a f", a=2),
                            et_t[:, p0:p0 + 2, :],
                            etq[:, jt % 4, :].rearrange(
                                "p (a f) -> p a f", a=2),
                            mybir.AluOpType.mult)
                    else:
                        for ihh, (et_t, p, _c) in enumerate(refs[:2]):
                            nc.vector.tensor_tensor(
                                t2[:, ihh * 512:(ihh + 1) * 512],
                                et_t[:, p, :],
                                etq[:, jt % 4, ihh * 512:(ihh + 1) * 512],
                                mybir.AluOpType.mult)

            def emit_slot(h, jt, b, et2):
                """Dots + exp + (deferred) E-mult for one (h, jt, b) slot."""
                refs = []
                for ih in range(2):
                    r = dots_chunk(h, jt, b, ih)
                    refs.append((r[0], r[1], cstate["c"] - 1))
                t2 = et2p.tile([128, N], BF16, tag="et2",
                               name=f"et2_{h}{jt}{b}")
                et2[jt, b] = t2
                cstate["pend"].append((h, jt, b, t2, refs))
                drain_mults()

            et2_prev = None
            for h in range(HEADS):
                et2 = {}
                # AV for head h-1 drains group-serial over this head's slots
                # ((b,isl) groups of 8 accumulation matmuls; only 2 PSUM
                # banks), each normalized right after its stop matmul.
                av_q = []
                if et2_prev is not None:
                    av_q = [(b2, isl, j2) for b2 in range(BLOC)
                            for isl in range(2) for j2 in range(8)][::-1]
                if h == 0:
                    order = [(jt, b) for b in range(BLOC) for jt in range(8)]
                else:
                    order = [(jt, b) for jt in range(8) for b in range(BLOC)]
                avrate = 2
                # h7: b0's AV groups drain in-loop once their et2 E-mults
                # are emitted (emission-order gating -- an AV piece emitted
                # before its producer TT reads uninitialized SBUF on HW)
                av7_q = [(0, isl, j2) for isl in range(2)
                         for j2 in range(8)][::-1] \
                    if h == HEADS - 1 else []
                seen_quads = set()
                for slot, (jt, b) in enumerate(order):
                    qd = (h, jt // 4)
                    if qd not in seen_quads:
                        seen_quads.add(qd)
                        issue_equad()
                    emit_slot(h, jt, b, et2)
                    if av_q:
                        for _ in range(avrate):
                            if av_q:
                                b2, isl, j2 = av_q.pop()
                                emit_av_piece(h - 1, b2, isl, j2, et2_prev)
                    elif extras:
                        extras.pop()()
                        if (slot == 0 or slot >= 4) and extras:
                            extras.pop()()
                    if not av_q and av7_q and slot >= 11:
                        for _ in range(4):
                            if av7_q and (h, av7_q[-1][2],
                                          av7_q[-1][0]) in mult_done:
                                b2, isl, j2 = av7_q.pop()
                                emit_av_piece(h, b2, isl, j2, et2)
                while av_q:
                    b2, isl, j2 = av_q.pop()
                    emit_av_piece(h - 1, b2, isl, j2, et2_prev)
                while et2_prev is None and extras:
                    extras.pop()()
                et2_prev = et2

            # tail: flush the partial exp tile, then drain the last head's
            # AV groups + norms
            flush_exp()
            h7 = HEADS - 1
            while av7_q:
                b2, isl, j2 = av7_q.pop()
                emit_av_piece(h7, b2, isl, j2, et2_prev)
            for isl in range(2):
                for jt in range(8):
                    emit_av_piece(h7, 1, isl, jt, et2_prev)

            # ---------------- GELU (+BN_v offset) + out projection ----------
            for b in range(BLOC):
                for hp in range(4):
                    gi = nc.scalar.activation(gt[b, hp][:], gt[b, hp][:], Gelu,
                                              bias=ovg_s[:, hp:hp + 1],
                                              scale=1.0)
                    if last_exp[0] is not None:
                        add_dep_helper(gi.ins, last_exp[0].ins, sync=False,
                                       reason="group ACT table sets")
                osb = persist.tile([128, 8, C], BF16, tag=f"osb{b}",
                                   name=f"osb{b}")
                for pr in range(4):
                    if pr % 2 == 1:
                        ops = dpsum.tile([128, NCHUNK, 512], F32, tag="dps",
                                         name=f"opd{b}{pr}")[:, 0, :]
                    else:
                        ops = avp.tile([128, 512], F32, tag="av",
                                       name=f"op{b}{pr}")
                    for half in range(2):
                        it = 2 * pr + half
                        for hp in range(4):
                            nc.tensor.matmul(
                                ops[:, half * C:half * C + C],
                                gt[b, hp][:, it * 128:(it + 1) * 128],
                                wo_s[:, hp, :],
                                start=(hp == 0), stop=(hp == 3),
                                skip_group_check=True,
                            )
                    nc.vector.tensor_tensor(
                        osb[:, 2 * pr:2 * pr + 2, :], ops[:],
                        bout_s[:].rearrange("p (a c) -> p a c", a=2),
                        mybir.AluOpType.add)
                    dma.dma_start(
                        out_d[b, 256 * pr:256 * (pr + 1)].rearrange(
                            "(t i) c -> i t c", t=2),
                        osb[:, 2 * pr:2 * pr + 2, :])

    nc.compile()
    return nc


def _host_prep(x, w_q, bn_q, w_k, bn_k, w_v, bn_v, w_out, b_out, bn_out,
               pos_table):
    """Fold BN into weights, build exp-bias table, shard across cores."""
    def fold(bn):
        g, b_, m, v = [np.asarray(a, np.float64) for a in bn]
        s = g / np.sqrt(v + EPS)
        return s, b_ - m * s

    sq, oq = fold(bn_q)
    sk, ok = fold(bn_k)
    sv, ov = fold(bn_v)
    so, oo = fold(bn_out)

    def wtile(w, s, ncols):
        w_eff = (np.asarray(w, np.float64) * s[None, :]).astype(np.float32)
        return np.ascontiguousarray(
            w_eff.reshape(-1, 128, ncols).transpose(1, 0, 2)).astype(NPBF16)

    # [128, ct, C] per q/k -> [dt, 128, qk, ct, 128]
    wqk = np.stack([wtile(w_q, sq, C), wtile(w_k, sk, C)], axis=1)
    wqk = np.ascontiguousarray(
        wqk.reshape(128, 2, 2, 2, 128).transpose(3, 0, 1, 2, 4))
    wv = wtile(w_v, sv, IDV)
    wo = wtile(w_out, so, C)

    oqk_t = np.ascontiguousarray(np.stack(
        [oq.astype(np.float32).reshape(2, 128).T,
         ok.astype(np.float32).reshape(2, 128).T], axis=1))
    ovg_t = np.ascontiguousarray(ov.astype(np.float32).reshape(4, 128).T)
    bout_eff = (np.asarray(b_out, np.float64) * so + oo).astype(np.float32)
    bout_t = np.ascontiguousarray(np.broadcast_to(
        np.concatenate([bout_eff, bout_eff]), (128, 2 * C)))

    # E[h, jt, j1, i] = exp(bias[j, i, h] / SCALE)
    r = np.arange(32)
    pos = np.stack(np.meshgrid(r, r, indexing="ij"), axis=-1).reshape(-1, 2)
    rel = np.abs(pos[:, None, :] - pos[None, :, :])
    idx = rel[..., 0] * 32 + rel[..., 1]                 # [j, i]
    bias = np.asarray(pos_table, np.float32)[idx]        # [j, i, 8]
    etab = np.exp(bias / SCALE).transpose(2, 0, 1)       # [8, j, i]
    etab = np.ascontiguousarray(
        etab.reshape(HEADS, 8, 128, N)).astype(NPBF16)

    x = np.asarray(x, np.float32).reshape(-1, N, C)      # [B, n, C]
    common = dict(wqk=wqk, wv=wv, wo=wo, oqk=oqk_t, ovg=ovg_t,
                  bout=bout_t, etab=etab)
    in_maps = []
    for c in range(NCORES):
        xl = x[c * BLOC:(c + 1) * BLOC]                  # [2, n, C]
        xtl = xl.transpose(0, 2, 1).reshape(BLOC, 2, 128, N).astype(NPBF16)
        in_maps.append(dict(common, xt=np.ascontiguousarray(xtl)))
    return in_maps


def kernel(**inputs):
    if "nc" not in _CACHE:
        _CACHE["nc"] = _build_nc()
    nc = _CACHE["nc"]
    in_maps = _host_prep(**inputs)
    res = run_bass_kernel_spmd(nc, in_maps, core_ids=list(range(NCORES)),
                               trace=bool(int(os.environ.get("KTRACE", "0"))))
    _CACHE["last_result"] = res
    outs = [res.results[c]["out"].reshape(BLOC, 32, 32, C)
            for c in range(NCORES)]
    return np.concatenate(outs, axis=0).astype(np.float32)


if __name__ == "__main__":
    nc = _build_nc()
    print("build + compile OK")



# revision 94
# speedup vs baseline: 1.0034x; 1.0034x over previous
"""Trainium2 Bass kernel: 8-head attention block (BN-folded projections,
relative-position bias, softmax, GELU + output projection).

Sharding: data-parallel over batch across 8 NeuronCores (2 batch elems/core).
All weights / bias tables replicated; no collectives.

Engine-balanced design for the TimelineSim cost model (matmuls charge
output free-size on PE.ENGINE; ACT charges free-size + ~143ns/instr;
engines execute their instruction streams in order):

  ACT is the pacing engine: exp of the 2*8*1024*1024 attention logits.
  To cut per-instruction overhead, dots psums are written as a global
  CHUNK STREAM: [128,512] chunks packed 3-per-psum-tile ([128,3,512],
  double-buffered = 6 of 8 PSUM banks), so each exp instruction covers
  1536 elements (86 exps instead of 128). E-mults still run per
  (jt, b) slot on DVE (bf16 2x), slicing across exp tiles when a slot
  straddles a tile boundary.
  - relative-position bias applied as a POST-EXP multiply by the
    precomputed table E = exp(bias/scale) on DVE.
  - softmax denominators come free from AV: stationary is V augmented
    with 64 ones columns, so the AV matmul stream yields AV on
    partitions 0:64 and the softmax sums on 64:128.
  - AV runs GROUP-SERIAL ((b,isl) groups of 8 accumulation matmuls on
    2 rotating PSUM banks), each group normalized (DVE recip + mult,
    partition-shifted) right at its stop matmul.
  - software pipelining: head h's AV groups are emitted inside head
    h+1's dots/exp loop; V projection and dt=1 Q/K projections ride
    the h0 loop as extra units; half the V psum->sbuf copies go
    through ACT (Copy shares the Exp table set) to fill PE-paced gaps.
  - out-projection runs in it-PAIRS: two accumulation groups write
    disjoint column halves of one PSUM bank (start=True only touches
    the addresses each group writes), so one bias-TT and one DMA
    serve two units -- halving the DVE TT serialization that paces
    the tail. Psums alternate between the avp pool and the dpsum
    pool (free after the last exp) so 4 pairs are in flight.
  - wqk[0] rides the ACT DMA queue so the first projection's inputs
    land a queue-slot earlier; everything else streams on SP.

  HW workaround: tile_position (96,0) is fatal (quadrant-3 bug), so
  heads 3/7's dots run as K=64 matmuls at (64,0) against a KT copy
  with the neighbouring head's rows zeroed.

  HW legality notes (the BIR verifier rejects, cost model accepts):
  AluOpType.divide on DVE TensorTensor; keep reciprocal+mult.
"""

import os
import numpy as np
import ml_dtypes

import concourse.bass as bass
import concourse.tile as tile
from concourse import bacc, mybir
from concourse.bass_utils import run_bass_kernel_spmd
from concourse.tile import add_dep_helper

NPBF16 = ml_dtypes.bfloat16
BF16 = mybir.dt.bfloat16
F32 = mybir.dt.float32

HEADS, DK, DV = 8, 32, 64
N = 1024          # positions = 32*32
C = 256           # channels
IDV = HEADS * DV  # 512
NCORES = 8
BLOC = 2          # batch elems per core
SCALE = float(DK) ** -0.5
EPS = 1e-5

_CACHE = {}


def _build_nc():
    nc = bacc.Bacc("TRN2", target_bir_lowering=False, debug=False)

    xt_d = nc.declare_dram_parameter("xt", [BLOC, 2, 128, N], BF16, isOutput=False)
    wqk_d = nc.declare_dram_parameter("wqk", [2, 128, 2, 2, 128], BF16,
                                      isOutput=False)
    wv_d = nc.declare_dram_parameter("wv", [128, 2, IDV], BF16, isOutput=False)
    wo_d = nc.declare_dram_parameter("wo", [128, 4, C], BF16, isOutput=False)
    oqk_d = nc.declare_dram_parameter("oqk", [128, 2, 2], F32, isOutput=False)
    ovg_d = nc.declare_dram_parameter("ovg", [128, 4], F32, isOutput=False)
    bout_d = nc.declare_dram_parameter("bout", [128, 2 * C], F32,
                                       isOutput=False)
    # E[h, jt, j1, i] = exp(pos_bias[j, i, h] / SCALE)
    e_d = nc.declare_dram_parameter("etab", [HEADS, 8, 128, N], BF16,
                                    isOutput=False)
    out_d = nc.declare_dram_parameter("out", [BLOC, N, C], BF16, isOutput=True)

    Exp = mybir.ActivationFunctionType.Exp
    Gelu = mybir.ActivationFunctionType.Gelu

    with tile.TileContext(nc) as tc:
        with (
            tc.tile_pool(name="const", bufs=1) as const,
            tc.tile_pool(name="persist", bufs=1) as persist,
            tc.tile_pool(name="ep", bufs=4) as ep,        # E tiles in flight
            tc.tile_pool(name="etp", bufs=8) as etp,      # raw exp tiles
            tc.tile_pool(name="et2p", bufs=22) as et2p,   # attn-weight tiles
            tc.tile_pool(name="recp", bufs=3) as recp,
            tc.tile_pool(name="dpsum", bufs=2, space="PSUM") as dpsum,
            tc.tile_pool(name="avp", bufs=2, space="PSUM") as avp,
        ):
            dma = nc.sync

            # critical-path DMAs first: b0's x, then the Q/K weights the
            # first dots need, then everything else.
            xt = {}
            for b in range(BLOC):
                for ct in range(2):
                    xt[b, ct] = persist.tile([128, N], BF16, tag=f"xt{b}{ct}",
                                             name=f"xt{b}{ct}")
            wqk_s = const.tile([128, 2, 2, 2, 128], BF16, tag="wqk")
            nc.scalar.dma_start(wqk_s[:, 0], wqk_d[0])
            dma.dma_start(xt[0, 0][:], xt_d[0, 0])
            dma.dma_start(xt[0, 1][:], xt_d[0, 1])
            oqk_s = const.tile([128, 2, 2], F32, tag="oqk")
            dma.dma_start(oqk_s[:], oqk_d[:])
            dma.dma_start(xt[1, 0][:], xt_d[1, 0])
            dma.dma_start(xt[1, 1][:], xt_d[1, 1])
            dma.dma_start(wqk_s[:, 1], wqk_d[1])
            wv_s = const.tile([128, 2, IDV], BF16, tag="wv")
            dma.dma_start(wv_s[:], wv_d[:])
            wo_s = const.tile([128, 4, C], BF16, tag="wo")
            dma.dma_start(wo_s[:], wo_d[:])
            ovg_s = const.tile([128, 4], F32, tag="ovg")
            dma.dma_start(ovg_s[:], ovg_d[:])
            bout_s = const.tile([128, 2 * C], F32, tag="bout")
            dma.dma_start(bout_s[:], bout_d[:])


            qt, kt, kzt, vt = {}, {}, {}, {}
            for b in range(BLOC):
                for dt in range(2):
                    qt[b, dt] = persist.tile([128, N], BF16, tag=f"qt{b}{dt}",
                                             name=f"qt{b}{dt}")
                    kt[b, dt] = persist.tile([128, N], BF16, tag=f"kt{b}{dt}",
                                             name=f"kt{b}{dt}")
                    kzt[b, dt] = persist.tile([128, N], BF16, tag=f"kz{b}{dt}",
                                              name=f"kz{b}{dt}")

            def qk_unit(b, dt, which):
                """One Q-or-K projection tile: 4 matmuls + DVE assembly.

                Uses the avp psum pool (idle during startup/h0) so the
                dots double-buffer in dpsum is never blocked."""
                for ih in range(2):
                    qk_half(b, dt, which, ih)

            def qk_half(b, dt, which, ih):
                w = 0 if which == "q" else 1
                dst = qt[b, dt] if which == "q" else kt[b, dt]
                ps = avp.tile([128, 512], F32, tag="av",
                              name=f"pps{b}{dt}{which}{ih}")
                for ct in range(2):
                    nc.tensor.matmul(
                        ps[:],
                        wqk_s[:, dt, w, ct, :],
                        xt[b, ct][:, ih * 512:(ih + 1) * 512],
                        start=(ct == 0), stop=(ct == 1),
                    )
                nc.vector.tensor_scalar_add(
                    dst[:, ih * 512:(ih + 1) * 512], ps[:],
                    oqk_s[:, w, dt:dt + 1])

            def kz_unit(b, dt):
                """kz = KT with head-2 rows zeroed (cheap all-SBUF copy);
                only needed once heads 3 (dt0) / 7 (dt1) come up."""
                kz = kzt[b, dt]
                nc.gpsimd.memset(kz[64:96, :], 0.0)
                nc.gpsimd.tensor_copy(kz[96:128, :], kt[b, dt][96:128, :])

            def v_unit(b, jt, act_copy=False):
                """One Vaug tile: [j, (h, V|ones 128)]; cols 64:128 = 1.0.

                act_copy routes the psum->sbuf copy through the ACT engine
                (Copy shares the Exp table set, so no table loads); used
                for half the tiles to relieve DVE during startup."""
                v = persist.tile([128, HEADS, 128], BF16, tag=f"v{b}{jt}",
                                 name=f"v{b}{jt}")
                vt[b, jt] = v
                nc.gpsimd.memset(v[:, :, 64:128], 1.0)
                ps = avp.tile([128, 512], F32, tag="av", name=f"vps{b}{jt}")
                for ct in range(2):
                    nc.tensor.matmul(
                        ps[:],
                        xt[b, ct][:, jt * 128:(jt + 1) * 128],
                        wv_s[:, ct, :],
                        start=(ct == 0), stop=(ct == 1),
                    )
                if act_copy:
                    nc.scalar.activation(
                        v[:, :, 0:64],
                        ps[:].rearrange("p (h d) -> p h d", h=HEADS),
                        mybir.ActivationFunctionType.Copy)
                else:
                    nc.vector.tensor_copy(
                        v[:, :, 0:64],
                        ps[:].rearrange("p (h d) -> p h d", h=HEADS))

            # warm the PE p-state during the initial DMA wait: ~3us of
            # matmuls on a zeroed scratch (results never read).
            wscr = persist.tile([128, 512], BF16, tag="wscr", name="wscr")
            nc.gpsimd.memset(wscr[:], 0.0)
            for w in range(6):
                wps = avp.tile([128, 512], F32, tag="av", name=f"warm{w}")
                nc.tensor.matmul(wps[:], wscr[:, 0:128], wscr[:],
                                 start=True, stop=True)

            # E-table quad DMAs, kept 2 in flight ahead of consumption
            equads = [(h, q) for h in range(HEADS) for q in range(2)]
            etabs = {}
            eptr = [0]

            def issue_equad():
                if eptr[0] >= len(equads):
                    return
                h, q = equads[eptr[0]]
                eptr[0] += 1
                t = ep.tile([128, 4, N], BF16, tag="etab", name=f"e{h}{q}")
                etabs[h, q] = t
                dma.dma_start(
                    t[:], e_d[h, 4 * q:4 * q + 4].rearrange("t j i -> j t i"))

            issue_equad()
            issue_equad()

            # b0's dt=0 projections up front (head 0 starts on them); all
            # other projection work rides head 0's loop as extra units.
            qk_half(0, 0, "q", 0)
            qk_half(0, 0, "k", 0)
            qk_half(0, 0, "q", 1)
            qk_half(0, 0, "k", 1)
            extras = [lambda: qk_unit(1, 0, "q"), lambda: qk_unit(1, 0, "k")]
            for b in range(BLOC):
                for jt in range(8):
                    extras.append(lambda b=b, jt=jt: v_unit(b, jt, jt % 2 == 0))
            for b in range(BLOC):
                extras.append(lambda b=b: qk_unit(b, 1, "q"))
                extras.append(lambda b=b: qk_unit(b, 1, "k"))
            for b in range(BLOC):
                for dt in range(2):
                    extras.append(lambda b=b, dt=dt: kz_unit(b, dt))
            extras = extras[::-1]

            # gt[b, hp]: gelu-layout attention output [(2 heads x 64 d), i]
            gt = {}
            for b in range(BLOC):
                for hp in range(4):
                    gt[b, hp] = persist.tile([128, N], BF16, tag=f"g{b}{hp}",
                                             name=f"g{b}{hp}")

            last_exp = [None]
            avt = {}

            def emit_av_piece(h, b, isl, jt, et2_prev):
                """One AV+sums accumulation matmul; group = (h,b,isl) over
                8 jt chunks, normalized immediately at the stop matmul."""
                if (h, b, isl) not in avt:
                    avt[h, b, isl] = avp.tile([128, 512], F32, tag="av",
                                              name=f"av{h}{b}{isl}")
                nc.tensor.matmul(
                    avt[h, b, isl][:],
                    vt[b, jt][:, h, :],
                    et2_prev[jt, b][:, isl * 512:(isl + 1) * 512],
                    start=(jt == 0), stop=(jt == 7),
                )
                if jt == 7:
                    emit_norm_piece(h, b, isl)
                    del avt[h, b, isl]

            def emit_norm_piece(h, b, isl):
                """softmax divide: gt rows = av(0:64) * 1/sums(64:128)."""
                tp = avt[h, b, isl]
                rec = recp.tile([64, 512], F32, tag="rec",
                                name=f"rec{h}{b}{isl}")
                nc.vector.reciprocal(rec[:], tp[64:128, :])
                nc.vector.tensor_tensor(
                    gt[b, h // 2][64 * (h % 2):64 * (h % 2) + 64,
                                  isl * 512:(isl + 1) * 512],
                    tp[0:64, :], rec[:],
                    mybir.AluOpType.mult)

            # ---- chunk-stream dots/exp machinery: dots psums are written as
            # a global stream of [128,512] chunks packed 3-per-psum-tile, so
            # each exp instruction covers 1536 elems (fewer ACT fixed costs).
            # E-mults still run per (jt, b) slot, slicing across exp tiles.
            cstate = {
                "c": 0,          # next chunk index
                "tile": None,    # current dps tile
                "et": None,      # current et tile
                "base": 0,       # chunk index of current tile's pos 0
                "exp_c": 0,      # chunks covered by emitted exps
                "pend": [],      # pending E-mult slots
            }
            NCHUNK = 3

            def dots_chunk(h, jt, b, ih):
                """Write one [128,512] dots chunk into the stream."""
                dt, hq = h // 4, h % 4
                pos = cstate["c"] % NCHUNK
                if pos == 0:
                    cstate["tile"] = dpsum.tile([128, NCHUNK, 512], F32,
                                                tag="dps",
                                                name=f"dps{cstate['c']}")
                    cstate["et"] = etp.tile([128, NCHUNK, 512], BF16,
                                            tag="et", name=f"et{cstate['c']}")
                    cstate["base"] = cstate["c"]
                dps = cstate["tile"]
                if hq < 3:
                    nc.tensor.matmul(
                        dps[:, pos, :],
                        kt[b, dt][32 * hq:32 * hq + 32,
                                  jt * 128:(jt + 1) * 128],
                        qt[b, dt][32 * hq:32 * hq + 32,
                                  ih * 512:(ih + 1) * 512],
                        start=True, stop=True,
                        tile_position=(32 * hq, 0),
                    )
                else:
                    nc.tensor.matmul(
                        dps[:, pos, :],
                        kzt[b, dt][64:128, jt * 128:(jt + 1) * 128],
                        qt[b, dt][64:128, ih * 512:(ih + 1) * 512],
                        start=True, stop=True,
                        tile_position=(64, 0),
                    )
                ref = (cstate["et"], pos)
                cstate["c"] += 1
                if cstate["c"] - cstate["base"] == NCHUNK:
                    flush_exp()
                return ref

            def flush_exp():
                """Exp the current (possibly partial) dps tile."""
                ntile = cstate["c"] - cstate["base"]
                if cstate["tile"] is None or ntile == 0:
                    return
                nc.scalar.activation(cstate["et"][:, 0:ntile, :],
                                     cstate["tile"][:, 0:ntile, :],
                                     Exp, scale=SCALE)
                # realign so the next chunk starts a fresh tile
                cstate["c"] = cstate["base"] + NCHUNK
                cstate["exp_c"] = cstate["c"]
                cstate["tile"] = None
                drain_mults()

            mult_done = set()

            def drain_mults():
                pend = cstate["pend"]
                while pend and pend[0][4][-1][2] < cstate["exp_c"]:
                    h, jt, b, t2, refs = pend.pop(0)
                    mult_done.add((h, jt, b))
                    qd = (h, jt // 4)
                    etq = etabs[qd]
                    if refs[0][0] is refs[1][0]:
                        et_t, p0 = refs[0][0], refs[0][1]
                        nc.vector.tensor_tensor(
                            t2[:].rearrange("p (a f) -> p a f", a=2),
                            et_t[:, p0:p0 + 2, :],
                            etq[:, jt % 4, :].rearrange(
                                "p (a f) -> p a f", a=2),
                            mybir.AluOpType.mult)
                    else:
                        for ihh, (et_t, p, _c) in enumerate(refs[:2]):
                            nc.vector.tensor_tensor(
                                t2[:, ihh * 512:(ihh + 1) * 512],
                                et_t[:, p, :],
                                etq[:, jt % 4, ihh * 512:(ihh + 1) * 512],
                                mybir.AluOpType.mult)

            def emit_slot(h, jt, b, et2):
                """Dots + exp + (deferred) E-mult for one (h, jt, b) slot."""
                refs = []
                for ih in range(2):
                    r = dots_chunk(h, jt, b, ih)
                    refs.append((r[0], r[1], cstate["c"] - 1))
                t2 = et2p.tile([128, N], BF16, tag="et2",
                               name=f"et2_{h}{jt}{b}")
                et2[jt, b] = t2
                cstate["pend"].append((h, jt, b, t2, refs))
                drain_mults()

            et2_prev = None
            for h in range(HEADS):
                et2 = {}
                # AV for head h-1 drains group-serial over this head's slots
                # ((b,isl) groups of 8 accumulation matmuls; only 2 PSUM
                # banks), each normalized right after its stop matmul.
                av_q = []
                if et2_prev is not None:
                    av_q = [(b2, isl, j2) for b2 in range(BLOC)
                            for isl in range(2) for j2 in range(8)][::-1]
                if h == 0:
                    order = [(jt, b) for b in range(BLOC) for jt in range(8)]
                else:
                    order = [(jt, b) for jt in range(8) for b in range(BLOC)]
                avrate = 2
                # h7: b0's AV groups drain in-loop once their et2 E-mults
                # are emitted (emission-order gating -- an AV piece emitted
                # before its producer TT reads uninitialized SBUF on HW)
                av7_q = [(0, isl, j2) for isl in range(2)
                         for j2 in range(8)][::-1] \
                    if h == HEADS - 1 else []
                seen_quads = set()
                for slot, (jt, b) in enumerate(order):
                    qd = (h, jt // 4)
                    if qd not in seen_quads:
                        seen_quads.add(qd)
                        issue_equad()
                    emit_slot(h, jt, b, et2)
                    if h == 0 and slot < 8:
                        flush_exp()
                    if av_q:
                        for _ in range(avrate):
                            if av_q:
                                b2, isl, j2 = av_q.pop()
                                emit_av_piece(h - 1, b2, isl, j2, et2_prev)
                    elif extras:
                        extras.pop()()
                        if (slot == 0 or slot >= 4) and extras:
                            extras.pop()()
                    if not av_q and av7_q and slot >= 11:
                        for _ in range(4):
                            if av7_q and (h, av7_q[-1][2],
                                          av7_q[-1][0]) in mult_done:
                                b2, isl, j2 = av7_q.pop()
                                emit_av_piece(h, b2, isl, j2, et2)
                while av_q:
                    b2, isl, j2 = av_q.pop()
                    emit_av_piece(h - 1, b2, isl, j2, et2_prev)
                while et2_prev is None and extras:
                    extras.pop()()
                et2_prev = et2

            # tail: flush the partial exp tile, then drain the last head's
            # AV groups + norms
            flush_exp()
            h7 = HEADS - 1
            while av7_q:
                b2, isl, j2 = av7_q.pop()
                emit_av_piece(h7, b2, isl, j2, et2_prev)
            for isl in range(2):
                for jt in range(8):
                    emit_av_piece(h7, 1, isl, jt, et2_prev)

            # ---------------- GELU (+BN_v offset) + out projection ----------
            for b in range(BLOC):
                for hp in range(4):
                    gi = nc.scalar.activation(gt[b, hp][:], gt[b, hp][:], Gelu,
                                              bias=ovg_s[:, hp:hp + 1],
                                              scale=1.0)
                    if last_exp[0] is not None:
                        add_dep_helper(gi.ins, last_exp[0].ins, sync=False,
                                       reason="group ACT table sets")
                osb = persist.tile([128, 8, C], BF16, tag=f"osb{b}",
                                   name=f"osb{b}")
                for pr in range(4):
                    if pr % 2 == 1:
                        ops = dpsum.tile([128, NCHUNK, 512], F32, tag="dps",
                                         name=f"opd{b}{pr}")[:, 0, :]
                    else:
                        ops = avp.tile([128, 512], F32, tag="av",
                                       name=f"op{b}{pr}")
                    for half in range(2):
                        it = 2 * pr + half
                        for hp in range(4):
                            nc.tensor.matmul(
                                ops[:, half * C:half * C + C],
                                gt[b, hp][:, it * 128:(it + 1) * 128],
                                wo_s[:, hp, :],
                                start=(hp == 0), stop=(hp == 3),
                                skip_group_check=True,
                            )
                    nc.vector.tensor_tensor(
                        osb[:, 2 * pr:2 * pr + 2, :], ops[:],
                        bout_s[:].rearrange("p (a c) -> p a c", a=2),
                        mybir.AluOpType.add)
                    dma.dma_start(
                        out_d[b, 256 * pr:256 * (pr + 1)].rearrange(
                            "(t i) c -> i t c", t=2),
                        osb[:, 2 * pr:2 * pr + 2, :])

    nc.compile()
    return nc


def _host_prep(x, w_q, bn_q, w_k, bn_k, w_v, bn_v, w_out, b_out, bn_out,
               pos_table):
    """Fold BN into weights, build exp-bias table, shard across cores."""
    def fold(bn):
        g, b_, m, v = [np.asarray(a, np.float64) for a in bn]
        s = g / np.sqrt(v + EPS)
        return s, b_ - m * s

    sq, oq = fold(bn_q)
    sk, ok = fold(bn_k)
    sv, ov = fold(bn_v)
    so, oo = fold(bn_out)

    def wtile(w, s, ncols):
        w_eff = (np.asarray(w, np.float64) * s[None, :]).astype(np.float32)
        return np.ascontiguousarray(
            w_eff.reshape(-1, 128, ncols).transpose(1, 0, 2)).astype(NPBF16)

    # [128, ct, C] per q/k -> [dt, 128, qk, ct, 128]
    wqk = np.stack([wtile(w_q, sq, C), wtile(w_k, sk, C)], axis=1)
    wqk = np.ascontiguousarray(
        wqk.reshape(128, 2, 2, 2, 128).transpose(3, 0, 1, 2, 4))
    wv = wtile(w_v, sv, IDV)
    wo = wtile(w_out, so, C)

    oqk_t = np.ascontiguousarray(np.stack(
        [oq.astype(np.float32).reshape(2, 128).T,
         ok.astype(np.float32).reshape(2, 128).T], axis=1))
    ovg_t = np.ascontiguousarray(ov.astype(np.float32).reshape(4, 128).T)
    bout_eff = (np.asarray(b_out, np.float64) * so + oo).astype(np.float32)
    bout_t = np.ascontiguousarray(np.broadcast_to(
        np.concatenate([bout_eff, bout_eff]), (128, 2 * C)))

    # E[h, jt, j1, i] = exp(bias[j, i, h] / SCALE)
    r = np.arange(32)
    pos = np.stack(np.meshgrid(r, r, indexing="ij"), axis=-1).reshape(-1, 2)
    rel = np.abs(pos[:, None, :] - pos[None, :, :])
    idx = rel[..., 0] * 32 + rel[..., 1]                 # [j, i]
    bias = np.asarray(pos_table, np.float32)[idx]        # [j, i, 8]
    etab = np.exp(bias / SCALE).transpose(2, 0, 1)       # [8, j, i]
    etab = np.ascontiguousarray(
        etab.reshape(HEADS, 8, 128, N)).astype(NPBF16)

    x = np.asarray(x, np.float32).reshape(-1, N, C)      # [B, n, C]
    common = dict(wqk=wqk, wv=wv, wo=wo, oqk=oqk_t, ovg=ovg_t,
                  bout=bout_t, etab=etab)
    in_maps = []
    for c in range(NCORES):
        xl = x[c * BLOC:(c + 1) * BLOC]                  # [2, n, C]
        xtl = xl.transpose(0, 2, 1).reshape(BLOC, 2, 128, N).astype(NPBF16)
        in_maps.append(dict(common, xt=np.ascontiguousarray(xtl)))
    return in_maps


def kernel(**inputs):
    if "nc" not in _CACHE:
        _CACHE["nc"] = _build_nc()
    nc = _CACHE["nc"]
    in_maps = _host_prep(**inputs)
    res = run_bass_kernel_spmd(nc, in_maps, core_ids=list(range(NCORES)),
                               trace=bool(int(os.environ.get("KTRACE", "0"))))
    _CACHE["last_result"] = res
    outs = [res.results[c]["out"].reshape(BLOC, 32, 32, C)
            for c in range(NCORES)]
    return np.concatenate(outs, axis=0).astype(np.float32)


if __name__ == "__main__":
    nc = _build_nc()
    print("build + compile OK")

